# revision 1
# baseline (speedup 1.0000x reference)
"""Multi-head graph attention network (GAT) Bass kernel for 8 Trainium2 NeuronCores.

Sharding: destination-node row-parallel (24 global blocks of 128 rows; core c
owns blocks 3c..3c+2 = 384 output rows). Edges are bucketed by destination
block on the host and padded to a uniform per-block count, so every core runs
an identical program on its own edge slice. No collectives: each core computes
its full output slice (incl. softmax, ELU, residual, LayerNorm) and the host
concatenates.
"""
import sys
sys.path.insert(0, '/opt/trn_rl_repo')

from contextlib import ExitStack

import numpy as np
import ml_dtypes

import concourse.bass as bass
import concourse.bacc as bacc
import concourse.tile as tile
from concourse import mybir
from concourse.bass_utils import run_bass_kernel_spmd

N = 3072
HID = 512
H = 8
HD = 64
E = 98304
LN_EPS = 1e-5
NCORES = 8
NBLK = 24            # global 128-row destination blocks
BPC = 3              # blocks per core
R = 128 * BPC        # rows per core
CHUNK = 512          # gather chunk (edges; transposed dma_gather breaks above 512)

f32 = mybir.dt.float32
bf16 = mybir.dt.bfloat16
Alu = mybir.AluOpType
Act = mybir.ActivationFunctionType


def _wrap_idx(idx):
    """int16 idx array -> [128, n/16] wrapped layout (edge k at row k%16,
    col k//16; 16-row pattern replicated to all 128 partitions)."""
    n = idx.shape[0]
    assert n % 16 == 0
    w16 = idx.reshape(n // 16, 16).T.astype(np.int16)
    return np.ascontiguousarray(np.tile(w16, (8, 1)))


def prepare(x, edges, Wv, bv, Ww, bw, Wa, ba, gamma, beta):
    """Host-side sharding/preprocessing. Returns (in_maps, B_pad, P)."""
    # % N matches jax negative-index semantics (x[-1] == x[N-1]); no-op for
    # well-formed inputs
    e0 = np.asarray(edges[0], np.int64) % N
    e1 = np.asarray(edges[1], np.int64) % N
    blk = e0 >> 7
    order = np.argsort(blk, kind="stable")
    counts = np.bincount(blk, minlength=NBLK)
    B_pad = max(128, int(-(-counts.max() // 128) * 128))
    P = BPC * B_pad

    ga_idx = np.zeros((NBLK, B_pad), np.int16)
    gb_idx = np.zeros((NBLK, B_pad), np.int16)
    onehot = np.zeros((NBLK, B_pad, 128), np.float32)
    starts = np.zeros(NBLK + 1, np.int64)
    starts[1:] = np.cumsum(counts)
    for b in range(NBLK):
        ids = order[starts[b]:starts[b + 1]]
        c = len(ids)
        ga_idx[b, :c] = e0[ids]
        ga_idx[b, c:] = b * 128          # pad: any valid row (zero onehot)
        gb_idx[b, :c] = e1[ids]
        onehot[b, np.arange(c), e0[ids] - b * 128] = 1.0

    x = np.asarray(x, np.float32)
    xT_bf = np.ascontiguousarray(x.T.astype(ml_dtypes.bfloat16))
    Wv = np.ascontiguousarray(np.asarray(Wv, np.float32))
    Ww = np.asarray(Ww, np.float32)
    wa_vec = np.asarray(Wa, np.float32).reshape(2 * HD)   # [128], shared by heads
    Wt_bf = np.ascontiguousarray(Ww[:HID].astype(ml_dtypes.bfloat16))
    Wb_bf = np.ascontiguousarray(Ww[HID:].astype(ml_dtypes.bfloat16))
    bv_b = np.ascontiguousarray(np.broadcast_to(bv, (128, HID)).astype(np.float32))
    bw_b = np.ascontiguousarray(np.broadcast_to(bw, (128, 2 * HID)).astype(np.float32))
    gamma_b = np.ascontiguousarray(np.broadcast_to(gamma, (128, HID)).astype(np.float32))
    beta_b = np.ascontiguousarray(np.broadcast_to(beta, (128, HID)).astype(np.float32))
    # per-head Wa dot as column-pattern lhsT in slot 0 (slot 1 unused)
    wa_lhsT = np.zeros((128, 2, H, H), np.float32)
    for h in range(H):
        wa_lhsT[:, 0, h, h] = wa_vec
    wa_lhsT = wa_lhsT.astype(ml_dtypes.bfloat16)

    in_maps = []
    for c in range(NCORES):
        bs = slice(BPC * c, BPC * (c + 1))
        in_maps.append(dict(
            xT=xT_bf,
            xs=np.ascontiguousarray(x[R * c:R * (c + 1)]),
            Wv=Wv.astype(ml_dtypes.bfloat16),
            Wt=Wt_bf,
            Wb=Wb_bf,
            bv_b=bv_b,
            bw_b=bw_b,
            gamma_b=gamma_b,
            beta_b=beta_b,
            wa_lhsT=np.ascontiguousarray(wa_lhsT.reshape(128, 2 * H * H)),
            ga_idx=_wrap_idx(ga_idx[bs].reshape(-1)),
            gb_idx=_wrap_idx(gb_idx[bs].reshape(-1)),
            onehot=np.ascontiguousarray(onehot[bs].reshape(P, 128).astype(ml_dtypes.bfloat16)),
        ))
    return in_maps, B_pad, P


def build(B_pad, P, dbg=99):
    nc = bacc.Bacc("TRN2", target_bir_lowering=False, num_devices=NCORES)

    xt_in = nc.dram_tensor("xT", [HID, N], bf16, kind="ExternalInput").ap()
    xs_in = nc.dram_tensor("xs", [R, HID], f32, kind="ExternalInput").ap()
    wv_in = nc.dram_tensor("Wv", [HID, HID], bf16, kind="ExternalInput").ap()
    wt_in = nc.dram_tensor("Wt", [HID, 2 * HID], bf16, kind="ExternalInput").ap()
    wb_in = nc.dram_tensor("Wb", [HID, 2 * HID], bf16, kind="ExternalInput").ap()
    bv_in = nc.dram_tensor("bv_b", [128, HID], f32, kind="ExternalInput").ap()
    bw_in = nc.dram_tensor("bw_b", [128, 2 * HID], f32, kind="ExternalInput").ap()
    gam_in = nc.dram_tensor("gamma_b", [128, HID], f32, kind="ExternalInput").ap()
    bet_in = nc.dram_tensor("beta_b", [128, HID], f32, kind="ExternalInput").ap()
    wa_in = nc.dram_tensor("wa_lhsT", [128, 2 * H * H], bf16, kind="ExternalInput").ap()
    gai_in = nc.dram_tensor("ga_idx", [128, P // 16], mybir.dt.int16, kind="ExternalInput").ap()
    gbi_in = nc.dram_tensor("gb_idx", [128, P // 16], mybir.dt.int16, kind="ExternalInput").ap()
    oh_in = nc.dram_tensor("onehot", [P, 128], bf16, kind="ExternalInput").ap()
    y_out = nc.dram_tensor("y", [R, HID], f32, kind="ExternalOutput").ap()

    a_tbl = nc.dram_tensor("a_tbl", [N, 2 * HID], bf16, kind="Internal").ap()
    b_tbl = nc.dram_tensor("b_tbl", [N, 2 * HID], bf16, kind="Internal").ap()
    v_tbl = nc.dram_tensor("v_tbl", [N, HID], bf16, kind="Internal").ap()

    NT = N // 128  # node tiles

    with TileKernel(nc) as tc, ExitStack() as ctx:
        const = ctx.enter_context(tc.tile_pool(name="const", bufs=1))

        # identity matrix for PE transpose, via iota + compare
        iota_row = const.tile([128, 128], mybir.dt.int32)
        nc.gpsimd.iota(iota_row[:], pattern=[[1, 128]], base=0, channel_multiplier=0)
        pid = const.tile([128, 1], mybir.dt.int32)
        nc.gpsimd.iota(pid[:], pattern=[[0, 1]], base=0, channel_multiplier=1)
        iota_f = const.tile([128, 128], f32)
        nc.vector.tensor_copy(iota_f[:], iota_row[:])
        pid_f = const.tile([128, 1], f32)
        nc.vector.tensor_copy(pid_f[:], pid[:])
        ident = const.tile([128, 128], f32)
        nc.vector.tensor_scalar(ident[:], iota_f[:], pid_f[:], None, op0=Alu.is_equal)

        gam_sb = const.tile([128, HID], f32)
        nc.sync.dma_start(gam_sb[:], gam_in)
        bet_sb = const.tile([128, HID], f32)
        nc.sync.dma_start(bet_sb[:], bet_in)
        wa_sb = const.tile([128, 2, H, H], bf16)
        nc.sync.dma_start(wa_sb[:], wa_in.rearrange("p (s a b) -> p s a b", s=2, a=H))
        xs_sb = const.tile([128, BPC, HID], f32)
        nc.sync.dma_start(xs_sb[:], xs_in.rearrange("(b p) d -> p b d", p=128))
        gai_sb = const.tile([128, P // 16], mybir.dt.int16)
        nc.sync.dma_start(gai_sb[:], gai_in)
        gbi_sb = const.tile([128, P // 16], mybir.dt.int16)
        nc.sync.dma_start(gbi_sb[:], gbi_in)

        # ---------------- Stage 0: projection tables ----------------
        with ExitStack() as s0:
            if dbg >= 1:
              wpool = s0.enter_context(tc.tile_pool(name="wpool", bufs=1))
              s0p = s0.enter_context(tc.tile_pool(name="s0p", bufs=3))
              psum_t = s0.enter_context(tc.tile_pool(name="psum_t", bufs=2, space="PSUM"))
              psum_mm = s0.enter_context(tc.tile_pool(name="psum_mm", bufs=2, space="PSUM"))

              wv_sb = wpool.tile([128, 4, HID], bf16)
              nc.sync.dma_start(wv_sb[:], wv_in.rearrange("(a p) d -> p a d", p=128))
              wt_sb = wpool.tile([128, 4, 2 * HID], bf16)
              nc.sync.dma_start(wt_sb[:], wt_in.rearrange("(a p) d -> p a d", p=128))
              wb_sb = wpool.tile([128, 4, 2 * HID], bf16)
              nc.sync.dma_start(wb_sb[:], wb_in.rearrange("(a p) d -> p a d", p=128))
              bv_sb = wpool.tile([128, HID], f32)
              nc.sync.dma_start(bv_sb[:], bv_in)
              bw_sb = wpool.tile([128, 2 * HID], f32)
              nc.sync.dma_start(bw_sb[:], bw_in)

              # a/b tables first (they gate the stage-1 gathers); v last so the
              # scheduler overlaps the v projection with early gathers.
              # xT is host-provided (pure layout transform of the x input).
              xt_b_all = wpool.tile([128, 4, N], bf16)
              nc.sync.dma_start(xt_b_all[:], xt_in.rearrange("(a p) n -> p a n", p=128))
              for nt in range(NT):
                  # a/b projections (bf16), two 512-wide halves each
                  for half in range(2):
                      hs = slice(half * HID, (half + 1) * HID)
                      psa = psum_mm.tile([128, HID], f32, tag="proj")
                      for kk in range(4):
                          nc.tensor.matmul(psa[:], xt_b_all[:, kk, nt * 128:(nt + 1) * 128], wt_sb[:, kk, hs],
                                           start=(kk == 0), stop=(kk == 3))
                      a_sb = s0p.tile([128, HID], bf16)
                      nc.vector.tensor_add(a_sb[:], psa[:], bw_sb[:, hs])
                      nc.sync.dma_start(a_tbl[nt * 128:(nt + 1) * 128, hs], a_sb[:])

                      psb = psum_mm.tile([128, HID], f32, tag="proj")
                      for kk in range(4):
                          nc.tensor.matmul(psb[:], xt_b_all[:, kk, nt * 128:(nt + 1) * 128], wb_sb[:, kk, hs],
                                           start=(kk == 0), stop=(kk == 3))
                      b_sb = s0p.tile([128, HID], bf16)
                      nc.scalar.copy(b_sb[:], psb[:])
                      nc.sync.dma_start(b_tbl[nt * 128:(nt + 1) * 128, hs], b_sb[:])
              for nt in range(NT):
                  psv = psum_mm.tile([128, HID], f32, tag="proj")
                  for kk in range(4):
                      nc.tensor.matmul(psv[:],
                                       xt_b_all[:, kk, nt * 128:(nt + 1) * 128],
                                       wv_sb[:, kk, :],
                                       start=(kk == 0), stop=(kk == 3))
                  v_sb = s0p.tile([128, HID], bf16)
                  nc.vector.tensor_add(v_sb[:], psv[:], bv_sb[:])
                  nc.sync.dma_start(v_tbl[nt * 128:(nt + 1) * 128, :], v_sb[:])

        # ---------------- Stages 1+2: per destination block ----------------
        s12 = ctx.enter_context(tc.tile_pool(name="s12", bufs=4))
        acc = ctx.enter_context(tc.tile_pool(name="acc", bufs=1, space="PSUM"))
        psum_w = ctx.enter_context(tc.tile_pool(name="psum_w", bufs=3, space="PSUM"))
        post = ctx.enter_context(tc.tile_pool(name="post", bufs=1))

        chunks = []
        off = 0
        while off < B_pad:
            c = min(CHUNK, B_pad - off)
            chunks.append((off, c))
            off += c

        for blk in range(BPC):
            psum_y = acc.tile([128, HID], f32, tag="psum_y")
            psum_d = acc.tile([128, H], f32, tag="psum_d")
            base = blk * B_pad
            for ci, (coff, C) in enumerate(chunks if dbg >= 2 else []):
                off = base + coff
                i0, i1 = off // 16, (off + C) // 16
                ga = s12.tile([128, H, C], bf16, tag="ga")
                nc.gpsimd.dma_gather(
                    out_ap=ga[:], in_ap=a_tbl, idxs_ap=gai_sb[:, i0:i1],
                    num_idxs=C, num_idxs_reg=C, elem_size=2 * HID, transpose=True)
                gb = s12.tile([128, H, C], bf16, tag="gb")
                nc.gpsimd.dma_gather(
                    out_ap=gb[:], in_ap=b_tbl, idxs_ap=gbi_sb[:, i0:i1],
                    num_idxs=C, num_idxs_reg=C, elem_size=2 * HID, transpose=True)
                nc.vector.tensor_add(ga[:], ga[:], gb[:])
                nc.vector.scalar_tensor_tensor(ga[:], ga[:], 0.01, ga[:],
                                               op0=Alu.mult, op1=Alu.max)
                # per-head dot -> w [8, C] in PSUM, in 512-col slices
                if dbg >= 3:
                    p_t = s12.tile([128, C // 128, H], bf16, tag="p_t")
                    for s0_ in range(0, C, 512):
                        S = min(512, C - s0_)
                        psw = psum_w.tile([H, S], f32, tag="psw")
                        for h in range(H):
                            nc.tensor.matmul(psw[:], wa_sb[:, 0, h, :],
                                             ga[:, h, s0_:s0_ + S],
                                             start=(h == 0), stop=(h == H - 1))
                        p8 = s12.tile([H, S], f32, tag="p8")
                        nc.scalar.activation(p8[:], psw[:], Act.Exp)
                        for g in range(S // 128):
                            pst = psum_w.tile([128, H], f32, tag="pst")
                            nc.tensor.transpose(pst[:], p8[:, g * 128:(g + 1) * 128],
                                                ident[:H, :H])
                            nc.vector.tensor_copy(p_t[:, (s0_ // 128) + g, :], pst[:])
                # v gather + payload
                if dbg >= 4:
                    gv = s12.tile([128, C // 128, HID], bf16, tag="gv")
                    nc.gpsimd.dma_gather(
                        out_ap=gv[:], in_ap=v_tbl, idxs_ap=gbi_sb[:, i0:i1],
                        num_idxs=C, num_idxs_reg=C, elem_size=HID)
                    pay = s12.tile([128, C // 128, HID + H], bf16, tag="pay")
                    nc.vector.tensor_mul(
                        pay[:, :, :HID].rearrange("p c (h d) -> p c h d", h=H),
                        gv[:].rearrange("p c (h d) -> p c h d", h=H),
                        p_t[:].unsqueeze(3).broadcast_to([128, C // 128, H, HD]))
                    nc.vector.tensor_copy(pay[:, :, HID:], p_t[:])
                    oh = s12.tile([128, C // 128, 128], bf16, tag="oh")
                    nc.sync.dma_start(
                        oh[:], oh_in[off:off + C, :].rearrange("(c p) r -> p c r", p=128))
                    first = ci == 0
                    last = ci == len(chunks) - 1
                    for g in range(C // 128):
                        st = first and g == 0
                        sp = last and g == C // 128 - 1
                        nc.tensor.matmul(psum_y[:], oh[:, g, :], pay[:, g, :HID],
                                         start=st, stop=sp, skip_group_check=True)
                        nc.tensor.matmul(psum_d[:], oh[:, g, :], pay[:, g, HID:],
                                         start=st, stop=sp, skip_group_check=True)

            # ---------------- post: divide, ELU, residual, LayerNorm ----------------
            if dbg >= 5:
                den = post.tile([128, H], f32, tag="den")
                nc.vector.tensor_scalar_add(den[:], psum_d[:], 1e-30)
                rden = post.tile([128, H], f32, tag="rden")
                nc.vector.reciprocal(rden[:], den[:])
                y1 = post.tile([128, HID], f32, tag="y1")
                nc.vector.tensor_mul(
                    y1[:].rearrange("p (h d) -> p h d", h=H),
                    psum_y[:].rearrange("p (h d) -> p h d", h=H),
                    rden[:].unsqueeze(2).broadcast_to([128, H, HD]))
                m1 = post.tile([128, HID], f32, tag="m1")
                nc.vector.tensor_scalar_max(m1[:], y1[:], 0.0)
                t1 = post.tile([128, HID], f32, tag="t1")
                nc.vector.tensor_scalar_min(t1[:], y1[:], 0.0)
                t2 = post.tile([128, HID], f32, tag="t2")
                nc.scalar.activation(t2[:], t1[:], Act.Exp)
                y3 = post.tile([128, HID], f32, tag="y3")
                nc.vector.scalar_tensor_tensor(y3[:], t2[:], -1.0, m1[:],
                                               op0=Alu.add, op1=Alu.add)
                nc.vector.tensor_add(y3[:], y3[:], xs_sb[:, blk, :])
                mu = post.tile([128, 1], f32, tag="mu")
                nc.vector.reduce_sum(mu[:], y3[:], axis=mybir.AxisListType.X)
                nc.vector.tensor_scalar_mul(mu[:], mu[:], 1.0 / HID)
                yc = post.tile([128, HID], f32, tag="yc")
                nc.vector.tensor_scalar(yc[:], y3[:], mu[:], None, op0=Alu.subtract)
                sq = post.tile([128, HID], f32, tag="sq")
                nc.vector.tensor_mul(sq[:], yc[:], yc[:])
                s2 = post.tile([128, 1], f32, tag="s2")
                nc.vector.reduce_sum(s2[:], sq[:], axis=mybir.AxisListType.X)
                var = post.tile([128, 1], f32, tag="var")
                nc.vector.tensor_scalar(var[:], s2[:], 1.0 / HID, LN_EPS,
                                        op0=Alu.mult, op1=Alu.add)
                sd = post.tile([128, 1], f32, tag="sd")
                nc.scalar.sqrt(sd[:], var[:])
                rstd = post.tile([128, 1], f32, tag="rstd")
                nc.vector.reciprocal(rstd[:], sd[:])
                yn = post.tile([128, HID], f32, tag="yn")
                nc.vector.tensor_scalar(yn[:], yc[:], rstd[:], None, op0=Alu.mult)
                yf = post.tile([128, HID], f32, tag="yf")
                nc.vector.tensor_mul(yf[:], yn[:], gam_sb[:])
                nc.vector.tensor_add(yf[:], yf[:], bet_sb[:])
                nc.sync.dma_start(y_out[blk * 128:(blk + 1) * 128, :], yf[:])
            else:
                nc.sync.dma_start(y_out[blk * 128:(blk + 1) * 128, :], xs_sb[:, blk, :])

    nc.compile()
    return nc


def TileKernel(nc):
    return tile.TileContext(nc)


_CACHE = {}


def get_nc(B_pad, P, dbg=99):
    key = (B_pad, P, dbg)
    if key not in _CACHE:
        _CACHE[key] = build(B_pad, P, dbg)
    return _CACHE[key]


def kernel(**inputs) -> np.ndarray:
    in_maps, B_pad, P = prepare(**inputs)
    nc = get_nc(B_pad, P)
    res = run_bass_kernel_spmd(nc, in_maps, core_ids=list(range(NCORES)))
    out = np.concatenate([r["y"] for r in res.results], axis=0)
    return out.astype(np.float32)


if __name__ == "__main__":
    import reference
    inputs = {k: np.asarray(v) for k, v in reference.setup_inputs().items()}
    got = kernel(**inputs)
    want = np.asarray(reference.reference(**inputs))
    err = np.abs(got - want).max() / (np.abs(want).max() + 1e-12)
    print("abs-max relative error:", err)

